# revision 55
# baseline (speedup 1.0000x reference)
"""Multi-head attention forward (B=2, T=2048, C=1024, H=16) on 8 trn2 cores.

Sharding: 2-way data parallel over batch x 4-way tensor parallel over heads
(Megatron-style). Core r handles batch r//4 and heads 4*(r%4)..4*(r%4)+3.
Each core computes Q/K/V projections for its heads, causal flash-style
attention in a transposed (S^T) layout, and its partial c_proj contribution
y_part^T = Wc[:, my_cols] @ o_part^T; partials are reduced on the host.

Final layout notes (352us baseline -> ~158us):
- Entire PE datapath is bf16: fp32-mode matmuls run half-rate under the
  trn2 PE power throttle, and bf16 halves DMA and LDWEIGHTS traffic.
- x^T is packed t-slice-major so each 512-t slice is one contiguous
  8KB-per-partition DMA; weights are packed so the first projection
  group's operands arrive first. Projections, attention windows and
  c_proj form a single continuous PE stream.
- Attention processes q in 512-wide windows. Both heads of a pair write
  one fused [128, 1024] score tile (the diagonal chunk's hh=1 block is
  left-shifted to keep the written region contiguous) so each key chunk
  needs ONE activation-engine exp; the activation engine runs nothing
  but exp, and PV trails S by two key chunks.
- The ACT exp throughput (~1.1us per key chunk) slightly exceeds the
  PE's attention work (~0.86us), so projection and c_proj matmuls are
  drip-fed into the attention streams as micro-tasks from a SEPARATE
  small PSUM pool (so they are never gated by exp drains), at most one
  per two key chunks; the overflow is emitted between blocks
  alternating across both PSUM pools.
- Softmax normalization: reciprocal_approx_fast (DVE) ->
  partition_broadcast (GPSIMD) -> multiply (DVE). The multiplies are
  deferred into the next block's stream so they never head-of-line
  block the DVE queue, and c_proj for window w is emitted a block late.
- Softmax is computed without max subtraction (scores are O(12), safe)
  and the denominator comes from an appended ones column in the PV
  stationary operand (V_aug [128, 65]); the ones column is memset on
  device (a strided DMA would fragment into 2-byte packets).
- PSUM budget (8 banks): score ring 2x2 banks, feeder pool 1 bank,
  oT accumulator ring 3x1 banks.
"""
import sys

sys.path.insert(0, "/opt/trn_rl_repo")
sys.path.insert(0, "/root/.axon_site")

import numpy as np
import ml_dtypes
import concourse.bacc as bacc
import concourse.mybir as mybir
from concourse import tile
from concourse.bass_utils import run_bass_kernel_spmd

_dt = mybir.dt
F32 = _dt.float32
BF16 = _dt.bfloat16
AF = mybir.ActivationFunctionType
ALU = mybir.AluOpType
_BF = ml_dtypes.bfloat16
I16 = _dt.int16
# bf16 Schraudolph exp: bf16_bits(round(A*s + B)) ~ exp(0.125*s)
SCHR_A = 128 * 0.125 * 1.4426950408889634
SCHR_B = 16248.5

B, T, C = 2, 2048, 1024
H, DH = 16, 64
N_CORES = 8
TP = 4              # tensor-parallel width (heads)
HPC = H // TP       # 4 heads per core
CPC = HPC * DH      # 256 channel dims per core
NCH = C // 128      # 8 contraction chunks of 128
W = 512             # q window width / t slice width
NW = T // W         # 4 windows
VSTRIDE = (T // 128) * (DH + 1)   # 16 chunks * 65 cols per head in vaug


def _build():
    nc = bacc.Bacc("TRN2", target_bir_lowering=False, debug=False,
                   num_devices=N_CORES)

    xt = nc.dram_tensor("xt", [128, NCH * T], BF16, kind="ExternalInput")
    wq = nc.dram_tensor("wq", [128, NCH * CPC], BF16, kind="ExternalInput")
    wk = nc.dram_tensor("wk", [128, NCH * CPC], BF16, kind="ExternalInput")
    wv = nc.dram_tensor("wv", [128, NCH * CPC], BF16, kind="ExternalInput")
    wc = nc.dram_tensor("wc", [128, 2 * C], BF16, kind="ExternalInput")
    msk = nc.dram_tensor("msk", [128, 128], F32, kind="ExternalInput")
    yt = nc.dram_tensor("yt", [128, NW * NCH * W], BF16, kind="ExternalOutput")

    with tile.TileContext(nc) as tc:
        with (
            tc.tile_pool(name="sb", bufs=1) as sb,
            tc.tile_pool(name="pt", bufs=6) as ptp,
            tc.tile_pool(name="bcp", bufs=2) as bcp,
            tc.tile_pool(name="yts", bufs=2) as ysb,
            tc.tile_pool(name="mm", bufs=2, space="PSUM") as psA,
            tc.tile_pool(name="fd", bufs=1, space="PSUM") as psB,
            tc.tile_pool(name="ot", bufs=3, space="PSUM") as psO,
        ):
            # ---- loads (ordered so compute can start early) -------------
            wq_t = sb.tile([128, NCH * CPC], BF16, tag="wq", name="wq_t")
            nc.sync.dma_start(wq_t[:, 0:C], wq[:, 0:C])
            xt_t = sb.tile([128, NCH * T], BF16, tag="xt", name="xt_t")
            Q4 = NCH * W // 4
            for q in range(4):
                nc.sync.dma_start(xt_t[:, Q4 * q: Q4 * (q + 1)],
                                  xt[:, Q4 * q: Q4 * (q + 1)])
            wk_t = sb.tile([128, NCH * CPC], BF16, tag="wk", name="wk_t")
            nc.sync.dma_start(wk_t[:, 0:C], wk[:, 0:C])
            msk_t = sb.tile([128, 128], F32, tag="msk", name="msk_t")
            nc.sync.dma_start(msk_t[:], msk[:])
            wv_t = sb.tile([128, NCH * CPC], BF16, tag="wv", name="wv_t")
            nc.sync.dma_start(wv_t[:], wv[:])
            vaug = sb.tile([128, HPC * VSTRIDE], BF16, tag="vaug", name="vaug")
            nc.vector.memset(vaug[:, 64::65], 1.0)
            nc.sync.dma_start(wq_t[:, C:2 * C], wq[:, C:2 * C])
            nc.sync.dma_start(wk_t[:, C:2 * C], wk[:, C:2 * C])
            for ts in range(1, 4):
                nc.sync.dma_start(xt_t[:, NCH * W * ts: NCH * W * (ts + 1)],
                                  xt[:, NCH * W * ts: NCH * W * (ts + 1)])
            wc_t = sb.tile([128, 2 * C], BF16, tag="wc", name="wc_t")
            nc.sync.dma_start(wc_t[:], wc[:])

            # persistent SBUF state
            # QTt cols [T*p, T*(p+1)): head pair p; head 2p at partitions
            # 0-63, head 2p+1 at partitions 64-127. Same for KTt.
            QTt = sb.tile([128, 2 * T], BF16, tag="qt", name="QTt")
            KTt = sb.tile([128, 2 * T], BF16, tag="kt", name="KTt")
            # oTs[p]: normalized o^T for heads 2p (rows 0-63), 2p+1 (64-127)
            oTs = [sb.tile([128, T], BF16, tag=f"ots{p}", name=f"oTs{p}") for p in range(2)]
            vaug_h = vaug.rearrange("p (h x) -> p h x", h=HPC)

            def xsl(ts, cc, o0, o1):
                """xt_t cols of t-slice ts, contraction chunk cc, t range [o0,o1)."""
                return xt_t[:, NCH * W * ts + W * cc + o0: NCH * W * ts + W * cc + o1]

            # ---- projection micro-tasks (fed into attention streams) ----
            # Each task takes the PSUM pool to allocate from.
            def proj_tasks(ts):
                def qk_task(w_t, dst, p):
                    def run(pool, eng=None):
                        ps = pool.tile([128, W], F32,
                                       tag="mm" if pool is psA else "fd",
                                       name="psqk1")
                        for cc in range(NCH):
                            nc.tensor.matmul(
                                ps[:],
                                w_t[:, C * p + 128 * cc: C * p + 128 * cc + 128],
                                xsl(ts, cc, 0, W),
                                start=(cc == 0), stop=(cc == NCH - 1),
                            )
                        d = dst[:, T * p + W * ts: T * p + W * ts + W]
                        (nc.scalar.copy if eng == 's' else nc.vector.tensor_copy)(d, ps[:])
                    return run

                def v_task(r):
                    def run(pool, eng=None):
                        ps = pool.tile([128, CPC], F32,
                                       tag="mm" if pool is psA else "fd",
                                       name="psv1")
                        for cc in range(NCH):
                            nc.tensor.matmul(
                                ps[:],
                                xsl(ts, cc, 128 * r, 128 * r + 128),
                                wv_t[:, CPC * cc: CPC * cc + CPC],
                                start=(cc == 0), stop=(cc == NCH - 1),
                            )
                        ki = 4 * ts + r
                        d = vaug_h[:, :, 65 * ki: 65 * ki + 64]
                        s = ps.rearrange("p (h j) -> p h j", h=HPC)
                        (nc.scalar.copy if eng == 's' else nc.vector.tensor_copy)(d, s)
                    return run

                return [qk_task(wq_t, QTt, 0), v_task(0),
                        qk_task(wq_t, QTt, 1), v_task(1),
                        qk_task(wk_t, KTt, 0), v_task(2),
                        qk_task(wk_t, KTt, 1), v_task(3)]

            # ---- attention matmul stream for one (window, head pair) ----
            # feed: list of PE micro-task closures interleaved into the
            # stream so the PE absorbs the ACT exp-throughput deficit.
            def attn_block(w, hp, feed=()):
                feed = list(feed)
                kmax = 4 * (w + 1)
                oT = [psO.tile([65, W], F32, tag="ot", name="oT") for _ in range(2)]

                def emit_pv(ki, off, pt):
                    mv = (pt[:, off:W], pt[:, W:2 * W - off])
                    for hh in range(2):
                        head = 2 * hp + hh
                        nc.tensor.matmul(
                            oT[hh][:, off:],
                            vaug[:, VSTRIDE * head + 65 * ki:
                                 VSTRIDE * head + 65 * ki + 65],
                            mv[hh],
                            start=(ki == 0), stop=(ki == kmax - 1),
                            skip_group_check=True,
                        )

                pend = []
                fed = 0
                for ki in range(kmax):
                    q0 = max(W * w, 128 * ki)
                    off = q0 - W * w
                    st = psA.tile([128, 2 * W], F32, tag="mm", name="st")
                    # hh=0 scores at [off:W), hh=1 left-shifted at [W:2W-off)
                    dsts = (st[:, off:W], st[:, W:2 * W - off])
                    for hh in range(2):
                        nc.tensor.matmul(
                            dsts[hh],
                            KTt[64 * hh:64 * hh + 64,
                                T * hp + 128 * ki:T * hp + 128 * ki + 128],
                            QTt[64 * hh:64 * hh + 64, T * hp + q0:T * hp + W * w + W],
                            start=True, stop=True,
                        )
                    if 128 * ki >= W * w:
                        nc.vector.tensor_add(
                            st[:, off:off + 128], st[:, off:off + 128], msk_t[:])
                        nc.vector.tensor_add(
                            st[:, W:W + 128], st[:, W:W + 128], msk_t[:])
                    pt = ptp.tile([128, 2 * W], BF16, tag="pt", name="pt")
                    nc.scalar.activation(pt[:, off:2 * W - off],
                                         st[:, off:2 * W - off], AF.Exp, scale=0.125)
                    pend.append((ki, off, pt))
                    if len(pend) == 3:
                        emit_pv(*pend.pop(0))
                    while feed and fed * kmax < (ki + 1) * len(feed):
                        feed.pop(0)()
                        fed += 1
                for task in feed:
                    task()
                for p in pend:
                    emit_pv(*p)
                return oT

            # ---- softmax normalization -----------------------------------
            # Front half (den copy on ACT, small DVE reciprocal, GPSIMD
            # broadcast) is emitted right after the block; the two DVE
            # multiplies are returned as closures fed into the NEXT block's
            # stream so they never head-of-line-block the DVE queue while
            # waiting on the broadcast.
            def norm_front(w, hp, oT):
                mults = []
                for hh in range(2):
                    den = bcp.tile([1, W], F32, tag="den", name="den")
                    nc.vector.tensor_copy(den[:], oT[hh][64:65, :])
                    rc = bcp.tile([1, W], F32, tag="rc", name="rc")
                    nc.vector.reciprocal_approx_fast(rc[:], den[:])
                    bcs = bcp.tile([64, W], F32, tag="bcs", name="bcs")
                    nc.gpsimd.partition_broadcast(bcs[:], rc[:])

                    def m(hh=hh, bcs=bcs):
                        nc.vector.tensor_tensor(
                            oTs[hp][64 * hh:64 * hh + 64, W * w:W * w + W],
                            oT[hh][0:64, :], bcs[:], ALU.mult)
                    mults.append(m)
                return mults

            # ---- partial c_proj micro-tasks for one q window ------------
            def cproj_tasks(w):
                yt_s = ysb.tile([128, NCH * W], BF16, tag="yt", name="yt_s")

                def dc_task(dc):
                    def run(pool, eng=None):
                        ps = pool.tile([128, W], F32,
                                       tag="mm" if pool is psA else "fd",
                                       name="pscp")
                        for cc in range(2):
                            nc.tensor.matmul(
                                ps[:],
                                wc_t[:, C * cc + 128 * dc: C * cc + 128 * dc + 128],
                                oTs[cc][:, W * w: W * w + W],
                                start=(cc == 0), stop=(cc == 1),
                            )
                        d = yt_s[:, W * dc: W * dc + W]
                        (nc.scalar.copy if eng == 's' else nc.vector.tensor_copy)(d, ps[:])
                        if dc % 2 == 1:
                            nc.sync.dma_start(
                                yt[:, NCH * W * w + W * (dc - 1):
                                   NCH * W * w + W * (dc + 1)],
                                yt_s[:, W * (dc - 1): W * (dc + 1)])
                    return run

                return [dc_task(dc) for dc in range(NCH)]

            # ---- fused schedule ----------------------------------------
            # Attention blocks are the backbone; projection slices for the
            # next window and c_proj for the previous one are drip-fed into
            # their matmul streams.
            def emit_batch(tasks, eng=None):
                for i, t in enumerate(tasks):
                    e = ('s' if i % 2 else None) if eng == 'alt' else eng
                    t(psA if i % 2 else psB, e)

            def wrap(tasks, eng=None):
                return [(lambda t=t: t(psB, eng)) for t in tasks]

            t0 = proj_tasks(0)
            # pair-0 projections + V up front so attention (0,0) starts ASAP;
            # pair-1 Q/K are fed into the (0,0) attention stream.
            for i, t in enumerate((t0[0], t0[4], t0[1], t0[3], t0[5], t0[7])):
                t(psA if i % 2 else psB)
            pend_p1 = [t0[2], t0[6]]
            cp_feed = []
            carry = []
            mults = []
            for w in range(NW):
                kcap = 2 * (w + 1)   # max in-attention psB feeds per block
                tasks = pend_p1 + (proj_tasks(w + 1) if w < NW - 1 else []) \
                    + carry + cp_feed
                pend_p1 = []
                carry = []
                if w == 2 and len(tasks) > 12:
                    # hold some c_proj tasks back for feed-starved window 3
                    carry = tasks[12:]
                    tasks = tasks[:12]
                half = min(kcap, (len(tasks) + 1) // 2)
                f1, f2 = tasks[:half], tasks[half:2 * half]
                rest = tasks[2 * half:]
                r1, r2 = rest[:len(rest) // 2], rest[len(rest) // 2:]
                eng = 's' if w < 2 else None
                oT = attn_block(w, 0, feed=mults + wrap(f1, eng))
                m0 = norm_front(w, 0, oT)
                emit_batch(r1, eng)
                oT = attn_block(w, 1, feed=m0 + wrap(f2, eng))
                mults = norm_front(w, 1, oT)
                emit_batch(r2, eng)
                cp_feed = cproj_tasks(w) if w < NW - 1 else []
            for m in mults:
                m()
            emit_batch(cproj_tasks(NW - 1), eng='alt')

    nc.compile()
    return nc


_NC = None


def _get_nc():
    global _NC
    if _NC is None:
        _NC = _build()
    return _NC


def _pack(a):
    """[K*128, n] -> [128, K*n] with row-chunk i at cols [n*i, n*(i+1))."""
    k = a.shape[0] // 128
    return np.ascontiguousarray(
        a.reshape(k, 128, a.shape[1]).transpose(1, 0, 2).reshape(128, -1))


def _pack_pmaj(Wsub):
    """W[hs:hs+CPC, :] -> [128, 2*C] with col = C*p + 128*cc + j holding
    W^T[128*cc + r, 128*p + j] (p-major so each half is one DMA)."""
    a = _pack(np.ascontiguousarray(Wsub.T))          # col = CPC*cc + d
    return np.ascontiguousarray(
        a.reshape(128, NCH, 2, 128).transpose(0, 2, 1, 3).reshape(128, -1))


def _pack_x(xb):
    """x[b] [T, C] -> x^T t-slice-major [128, NW * NCH * W].

    col = NCH*W*ts + W*cc + t holds x^T[128*cc + p, W*ts + t].
    """
    xp = np.ascontiguousarray(xb.T)            # [C, T]
    return np.ascontiguousarray(
        xp.reshape(NCH, 128, NW, W).transpose(1, 2, 0, 3).reshape(128, -1))


def make_in_maps(x, Wq, Wk, Wv, Wc):
    x = np.asarray(x, np.float32)
    Wq, Wk, Wv, Wc = (np.asarray(w, np.float32) for w in (Wq, Wk, Wv, Wc))
    a = np.arange(128)
    msk = np.where(a[:, None] > a[None, :], np.float32(-1e9), np.float32(0.0))
    xt_b = [_pack_x(x[b]).astype(_BF) for b in range(B)]
    maps = []
    for r in range(N_CORES):
        b, rho = r // TP, r % TP
        hs = CPC * rho
        maps.append({
            "xt": xt_b[b],
            "wq": _pack_pmaj(Wq[hs:hs + CPC, :]).astype(_BF),
            "wk": _pack_pmaj(Wk[hs:hs + CPC, :]).astype(_BF),
            "wv": _pack(np.ascontiguousarray(Wv[hs:hs + CPC, :].T)).astype(_BF),
            "wc": _pack(np.ascontiguousarray(Wc[:, hs:hs + CPC].T)).astype(_BF),
            "msk": msk,
        })
    return maps


def assemble(results, bc):
    bc = np.asarray(bc, np.float32)
    outs = []
    for b in range(B):
        ysum = None
        for rho in range(TP):
            ytp = results[TP * b + rho]["yt"].astype(np.float32)
            y = ytp.reshape(128, NW, NCH, W).transpose(2, 0, 1, 3).reshape(C, T)
            ysum = y if ysum is None else ysum + y
        outs.append(ysum.T + bc[None, :])
    return np.stack(outs).astype(np.float32)


def kernel(x, Wq, Wk, Wv, Wc, bc, _run_kwargs=None):
    nc = _get_nc()
    in_maps = make_in_maps(x, Wq, Wk, Wv, Wc)
    res = run_bass_kernel_spmd(nc, in_maps, core_ids=list(range(N_CORES)),
                               **(_run_kwargs or {}))
    out = assemble(res.results, bc)
    kernel.last_results = res
    return out


# revision 56
# speedup vs baseline: 1.0115x; 1.0115x over previous
"""Multi-head attention forward (B=2, T=2048, C=1024, H=16) on 8 trn2 cores.

Sharding: 2-way data parallel over batch x 4-way tensor parallel over heads
(Megatron-style). Core r handles batch r//4 and heads 4*(r%4)..4*(r%4)+3.
Each core computes Q/K/V projections for its heads, causal flash-style
attention in a transposed (S^T) layout, and its partial c_proj contribution
y_part^T = Wc[:, my_cols] @ o_part^T; partials are reduced on the host.

Final layout notes (352us baseline -> ~158us):
- Entire PE datapath is bf16: fp32-mode matmuls run half-rate under the
  trn2 PE power throttle, and bf16 halves DMA and LDWEIGHTS traffic.
- x^T is packed t-slice-major so each 512-t slice is one contiguous
  8KB-per-partition DMA; weights are packed so the first projection
  group's operands arrive first. Projections, attention windows and
  c_proj form a single continuous PE stream.
- Attention processes q in 512-wide windows. Both heads of a pair write
  one fused [128, 1024] score tile (the diagonal chunk's hh=1 block is
  left-shifted to keep the written region contiguous) so each key chunk
  needs ONE activation-engine exp; the activation engine runs nothing
  but exp, and PV trails S by two key chunks.
- The ACT exp throughput (~1.1us per key chunk) slightly exceeds the
  PE's attention work (~0.86us), so projection and c_proj matmuls are
  drip-fed into the attention streams as micro-tasks from a SEPARATE
  small PSUM pool (so they are never gated by exp drains), at most one
  per two key chunks; the overflow is emitted between blocks
  alternating across both PSUM pools.
- Softmax normalization: reciprocal_approx_fast (DVE) ->
  partition_broadcast (GPSIMD) -> multiply (DVE). The multiplies are
  deferred into the next block's stream so they never head-of-line
  block the DVE queue, and c_proj for window w is emitted a block late.
- Softmax is computed without max subtraction (scores are O(12), safe)
  and the denominator comes from an appended ones column in the PV
  stationary operand (V_aug [128, 65]); the ones column is memset on
  device (a strided DMA would fragment into 2-byte packets).
- PSUM budget (8 banks): score ring 2x2 banks, feeder pool 1 bank,
  oT accumulator ring 3x1 banks.
"""
import sys

sys.path.insert(0, "/opt/trn_rl_repo")
sys.path.insert(0, "/root/.axon_site")

import numpy as np
import ml_dtypes
import concourse.bacc as bacc
import concourse.mybir as mybir
from concourse import tile
from concourse.bass_utils import run_bass_kernel_spmd

_dt = mybir.dt
F32 = _dt.float32
BF16 = _dt.bfloat16
AF = mybir.ActivationFunctionType
ALU = mybir.AluOpType
_BF = ml_dtypes.bfloat16

B, T, C = 2, 2048, 1024
H, DH = 16, 64
N_CORES = 8
TP = 4              # tensor-parallel width (heads)
HPC = H // TP       # 4 heads per core
CPC = HPC * DH      # 256 channel dims per core
NCH = C // 128      # 8 contraction chunks of 128
W = 512             # q window width / t slice width
NW = T // W         # 4 windows
VSTRIDE = (T // 128) * (DH + 1)   # 16 chunks * 65 cols per head in vaug


def _build():
    nc = bacc.Bacc("TRN2", target_bir_lowering=False, debug=False,
                   num_devices=N_CORES)

    xt = nc.dram_tensor("xt", [128, NCH * T], BF16, kind="ExternalInput")
    wq = nc.dram_tensor("wq", [128, NCH * CPC], BF16, kind="ExternalInput")
    wk = nc.dram_tensor("wk", [128, NCH * CPC], BF16, kind="ExternalInput")
    wv = nc.dram_tensor("wv", [128, NCH * CPC], BF16, kind="ExternalInput")
    wc = nc.dram_tensor("wc", [128, 2 * C], BF16, kind="ExternalInput")
    msk = nc.dram_tensor("msk", [128, 128], F32, kind="ExternalInput")
    yt = nc.dram_tensor("yt", [128, NW * NCH * W], BF16, kind="ExternalOutput")

    with tile.TileContext(nc) as tc:
        with (
            tc.tile_pool(name="sb", bufs=1) as sb,
            tc.tile_pool(name="pt", bufs=6) as ptp,
            tc.tile_pool(name="bcp", bufs=2) as bcp,
            tc.tile_pool(name="yts", bufs=2) as ysb,
            tc.tile_pool(name="mm", bufs=2, space="PSUM") as psA,
            tc.tile_pool(name="fd", bufs=1, space="PSUM") as psB,
            tc.tile_pool(name="ot", bufs=3, space="PSUM") as psO,
        ):
            # ---- loads (ordered so compute can start early) -------------
            wq_t = sb.tile([128, NCH * CPC], BF16, tag="wq", name="wq_t")
            nc.sync.dma_start(wq_t[:, 0:C], wq[:, 0:C])
            xt_t = sb.tile([128, NCH * T], BF16, tag="xt", name="xt_t")
            Q4 = NCH * W // 4
            for q in range(4):
                nc.sync.dma_start(xt_t[:, Q4 * q: Q4 * (q + 1)],
                                  xt[:, Q4 * q: Q4 * (q + 1)])
            wk_t = sb.tile([128, NCH * CPC], BF16, tag="wk", name="wk_t")
            nc.sync.dma_start(wk_t[:, 0:C], wk[:, 0:C])
            msk_t = sb.tile([128, 128], F32, tag="msk", name="msk_t")
            nc.sync.dma_start(msk_t[:], msk[:])
            wv_t = sb.tile([128, NCH * CPC], BF16, tag="wv", name="wv_t")
            nc.sync.dma_start(wv_t[:], wv[:])
            vaug = sb.tile([128, HPC * VSTRIDE], BF16, tag="vaug", name="vaug")
            nc.vector.memset(vaug[:, 64::65], 1.0)
            nc.sync.dma_start(wq_t[:, C:2 * C], wq[:, C:2 * C])
            nc.sync.dma_start(wk_t[:, C:2 * C], wk[:, C:2 * C])
            for ts in range(1, 4):
                nc.sync.dma_start(xt_t[:, NCH * W * ts: NCH * W * (ts + 1)],
                                  xt[:, NCH * W * ts: NCH * W * (ts + 1)])
            wc_t = sb.tile([128, 2 * C], BF16, tag="wc", name="wc_t")
            nc.sync.dma_start(wc_t[:], wc[:])

            # persistent SBUF state
            # QTt cols [T*p, T*(p+1)): head pair p; head 2p at partitions
            # 0-63, head 2p+1 at partitions 64-127. Same for KTt.
            QTt = sb.tile([128, 2 * T], BF16, tag="qt", name="QTt")
            KTt = sb.tile([128, 2 * T], BF16, tag="kt", name="KTt")
            # oTs[p]: normalized o^T for heads 2p (rows 0-63), 2p+1 (64-127)
            oTs = [sb.tile([128, T], BF16, tag=f"ots{p}", name=f"oTs{p}") for p in range(2)]
            vaug_h = vaug.rearrange("p (h x) -> p h x", h=HPC)

            def xsl(ts, cc, o0, o1):
                """xt_t cols of t-slice ts, contraction chunk cc, t range [o0,o1)."""
                return xt_t[:, NCH * W * ts + W * cc + o0: NCH * W * ts + W * cc + o1]

            # ---- projection micro-tasks (fed into attention streams) ----
            # Each task takes the PSUM pool to allocate from.
            def proj_tasks(ts):
                def qk_task(w_t, dst, p):
                    def run(pool, eng=None):
                        ps = pool.tile([128, W], F32,
                                       tag="mm" if pool is psA else "fd",
                                       name="psqk1")
                        for cc in range(NCH):
                            nc.tensor.matmul(
                                ps[:],
                                w_t[:, C * p + 128 * cc: C * p + 128 * cc + 128],
                                xsl(ts, cc, 0, W),
                                start=(cc == 0), stop=(cc == NCH - 1),
                            )
                        d = dst[:, T * p + W * ts: T * p + W * ts + W]
                        (nc.scalar.copy if eng == 's' else nc.vector.tensor_copy)(d, ps[:])
                    return run

                def v_task(r):
                    def run(pool, eng=None):
                        ps = pool.tile([128, CPC], F32,
                                       tag="mm" if pool is psA else "fd",
                                       name="psv1")
                        for cc in range(NCH):
                            nc.tensor.matmul(
                                ps[:],
                                xsl(ts, cc, 128 * r, 128 * r + 128),
                                wv_t[:, CPC * cc: CPC * cc + CPC],
                                start=(cc == 0), stop=(cc == NCH - 1),
                            )
                        ki = 4 * ts + r
                        d = vaug_h[:, :, 65 * ki: 65 * ki + 64]
                        s = ps.rearrange("p (h j) -> p h j", h=HPC)
                        (nc.scalar.copy if eng == 's' else nc.vector.tensor_copy)(d, s)
                    return run

                return [qk_task(wq_t, QTt, 0), v_task(0),
                        qk_task(wq_t, QTt, 1), v_task(1),
                        qk_task(wk_t, KTt, 0), v_task(2),
                        qk_task(wk_t, KTt, 1), v_task(3)]

            # ---- attention matmul stream for one (window, head pair) ----
            # feed: list of PE micro-task closures interleaved into the
            # stream so the PE absorbs the ACT exp-throughput deficit.
            def attn_block(w, hp, feed=()):
                feed = list(feed)
                kmax = 4 * (w + 1)
                oT = [psO.tile([65, W], F32, tag="ot", name="oT") for _ in range(2)]

                def emit_pv(ki, off, pt):
                    mv = (pt[:, off:W], pt[:, W:2 * W - off])
                    for hh in range(2):
                        head = 2 * hp + hh
                        nc.tensor.matmul(
                            oT[hh][:, off:],
                            vaug[:, VSTRIDE * head + 65 * ki:
                                 VSTRIDE * head + 65 * ki + 65],
                            mv[hh],
                            start=(ki == 0), stop=(ki == kmax - 1),
                            skip_group_check=True,
                        )

                pend = []
                fed = 0
                for ki in range(kmax):
                    q0 = max(W * w, 128 * ki)
                    off = q0 - W * w
                    st = psA.tile([128, 2 * W], F32, tag="mm", name="st")
                    # hh=0 scores at [off:W), hh=1 left-shifted at [W:2W-off)
                    dsts = (st[:, off:W], st[:, W:2 * W - off])
                    for hh in range(2):
                        nc.tensor.matmul(
                            dsts[hh],
                            KTt[64 * hh:64 * hh + 64,
                                T * hp + 128 * ki:T * hp + 128 * ki + 128],
                            QTt[64 * hh:64 * hh + 64, T * hp + q0:T * hp + W * w + W],
                            start=True, stop=True,
                        )
                    if 128 * ki >= W * w:
                        nc.vector.tensor_add(
                            st[:, off:off + 128], st[:, off:off + 128], msk_t[:])
                        nc.vector.tensor_add(
                            st[:, W:W + 128], st[:, W:W + 128], msk_t[:])
                    pt = ptp.tile([128, 2 * W], BF16, tag="pt", name="pt")
                    nc.scalar.activation(pt[:, off:2 * W - off],
                                         st[:, off:2 * W - off], AF.Exp, scale=0.125)
                    pend.append((ki, off, pt))
                    if len(pend) == 3:
                        emit_pv(*pend.pop(0))
                    while feed and fed * kmax < (ki + 1) * len(feed):
                        feed.pop(0)()
                        fed += 1
                for task in feed:
                    task()
                for p in pend:
                    emit_pv(*p)
                return oT

            # ---- softmax normalization -----------------------------------
            # Front half (den copy on ACT, small DVE reciprocal, GPSIMD
            # broadcast) is emitted right after the block; the two DVE
            # multiplies are returned as closures fed into the NEXT block's
            # stream so they never head-of-line-block the DVE queue while
            # waiting on the broadcast.
            def norm_front(w, hp, oT):
                mults = []
                for hh in range(2):
                    den = bcp.tile([1, W], F32, tag="den", name="den")
                    nc.vector.tensor_copy(den[:], oT[hh][64:65, :])
                    rc = bcp.tile([1, W], F32, tag="rc", name="rc")
                    nc.vector.reciprocal_approx_fast(rc[:], den[:])
                    bcs = bcp.tile([64, W], F32, tag="bcs", name="bcs")
                    nc.gpsimd.partition_broadcast(bcs[:], rc[:])

                    def m(hh=hh, bcs=bcs):
                        nc.vector.tensor_tensor(
                            oTs[hp][64 * hh:64 * hh + 64, W * w:W * w + W],
                            oT[hh][0:64, :], bcs[:], ALU.mult)
                    mults.append(m)
                return mults

            # ---- partial c_proj micro-tasks for one q window ------------
            def cproj_tasks(w):
                yt_s = ysb.tile([128, NCH * W], BF16, tag="yt", name="yt_s")

                def dc_task(dc):
                    def run(pool, eng=None):
                        ps = pool.tile([128, W], F32,
                                       tag="mm" if pool is psA else "fd",
                                       name="pscp")
                        for cc in range(2):
                            nc.tensor.matmul(
                                ps[:],
                                wc_t[:, C * cc + 128 * dc: C * cc + 128 * dc + 128],
                                oTs[cc][:, W * w: W * w + W],
                                start=(cc == 0), stop=(cc == 1),
                            )
                        d = yt_s[:, W * dc: W * dc + W]
                        (nc.scalar.copy if eng == 's' else nc.vector.tensor_copy)(d, ps[:])
                        if dc % 2 == 1:
                            nc.sync.dma_start(
                                yt[:, NCH * W * w + W * (dc - 1):
                                   NCH * W * w + W * (dc + 1)],
                                yt_s[:, W * (dc - 1): W * (dc + 1)])
                    return run

                return [dc_task(dc) for dc in range(NCH)]

            # ---- fused schedule ----------------------------------------
            # Attention blocks are the backbone; projection slices for the
            # next window and c_proj for the previous one are drip-fed into
            # their matmul streams.
            def emit_batch(tasks, eng=None):
                for i, t in enumerate(tasks):
                    e = ('s' if i % 2 else None) if eng == 'alt' else eng
                    t(psA if i % 2 else psB, e)

            def wrap(tasks, eng=None):
                return [(lambda t=t: t(psB, eng)) for t in tasks]

            t0 = proj_tasks(0)
            # pair-0 projections + V up front so attention (0,0) starts ASAP;
            # pair-1 Q/K are fed into the (0,0) attention stream.
            for i, t in enumerate((t0[0], t0[4], t0[1], t0[3], t0[5], t0[7])):
                t(psA if i % 2 else psB)
            pend_p1 = [t0[2], t0[6]]
            cp_feed = []
            carry = []
            mults = []
            for w in range(NW):
                kcap = 2 * (w + 1)   # max in-attention psB feeds per block
                tasks = pend_p1 + (proj_tasks(w + 1) if w < NW - 1 else []) \
                    + carry + cp_feed
                pend_p1 = []
                carry = []
                if w == 2 and len(tasks) > 12:
                    # hold some c_proj tasks back for feed-starved window 3
                    carry = tasks[12:]
                    tasks = tasks[:12]
                half = min(kcap, (len(tasks) + 1) // 2)
                f1, f2 = tasks[:half], tasks[half:2 * half]
                rest = tasks[2 * half:]
                r1, r2 = rest[:len(rest) // 2], rest[len(rest) // 2:]
                eng = 's' if w < 2 else None
                oT = attn_block(w, 0, feed=mults + wrap(f1, eng))
                m0 = norm_front(w, 0, oT)
                emit_batch(r1, eng)
                oT = attn_block(w, 1, feed=m0 + wrap(f2, eng))
                mults = norm_front(w, 1, oT)
                emit_batch(r2, eng)
                cp_feed = cproj_tasks(w) if w < NW - 1 else []
            for m in mults:
                m()
            emit_batch(cproj_tasks(NW - 1), eng='alt')

    nc.compile()
    return nc


_NC = None


def _get_nc():
    global _NC
    if _NC is None:
        _NC = _build()
    return _NC


def _pack(a):
    """[K*128, n] -> [128, K*n] with row-chunk i at cols [n*i, n*(i+1))."""
    k = a.shape[0] // 128
    return np.ascontiguousarray(
        a.reshape(k, 128, a.shape[1]).transpose(1, 0, 2).reshape(128, -1))


def _pack_pmaj(Wsub):
    """W[hs:hs+CPC, :] -> [128, 2*C] with col = C*p + 128*cc + j holding
    W^T[128*cc + r, 128*p + j] (p-major so each half is one DMA)."""
    a = _pack(np.ascontiguousarray(Wsub.T))          # col = CPC*cc + d
    return np.ascontiguousarray(
        a.reshape(128, NCH, 2, 128).transpose(0, 2, 1, 3).reshape(128, -1))


def _pack_x(xb):
    """x[b] [T, C] -> x^T t-slice-major [128, NW * NCH * W].

    col = NCH*W*ts + W*cc + t holds x^T[128*cc + p, W*ts + t].
    """
    xp = np.ascontiguousarray(xb.T)            # [C, T]
    return np.ascontiguousarray(
        xp.reshape(NCH, 128, NW, W).transpose(1, 2, 0, 3).reshape(128, -1))


def make_in_maps(x, Wq, Wk, Wv, Wc):
    x = np.asarray(x, np.float32)
    Wq, Wk, Wv, Wc = (np.asarray(w, np.float32) for w in (Wq, Wk, Wv, Wc))
    a = np.arange(128)
    msk = np.where(a[:, None] > a[None, :], np.float32(-1e9), np.float32(0.0))
    xt_b = [_pack_x(x[b]).astype(_BF) for b in range(B)]
    maps = []
    for r in range(N_CORES):
        b, rho = r // TP, r % TP
        hs = CPC * rho
        maps.append({
            "xt": xt_b[b],
            "wq": _pack_pmaj(Wq[hs:hs + CPC, :]).astype(_BF),
            "wk": _pack_pmaj(Wk[hs:hs + CPC, :]).astype(_BF),
            "wv": _pack(np.ascontiguousarray(Wv[hs:hs + CPC, :].T)).astype(_BF),
            "wc": _pack(np.ascontiguousarray(Wc[:, hs:hs + CPC].T)).astype(_BF),
            "msk": msk,
        })
    return maps


def assemble(results, bc):
    bc = np.asarray(bc, np.float32)
    outs = []
    for b in range(B):
        ysum = None
        for rho in range(TP):
            ytp = results[TP * b + rho]["yt"].astype(np.float32)
            y = ytp.reshape(128, NW, NCH, W).transpose(2, 0, 1, 3).reshape(C, T)
            ysum = y if ysum is None else ysum + y
        outs.append(ysum.T + bc[None, :])
    return np.stack(outs).astype(np.float32)


def kernel(x, Wq, Wk, Wv, Wc, bc, _run_kwargs=None):
    nc = _get_nc()
    in_maps = make_in_maps(x, Wq, Wk, Wv, Wc)
    res = run_bass_kernel_spmd(nc, in_maps, core_ids=list(range(N_CORES)),
                               **(_run_kwargs or {}))
    out = assemble(res.results, bc)
    kernel.last_results = res
    return out
